# revision 38
# baseline (speedup 1.0000x reference)
"""Bayesian categorical embedding lookup on 8 trn2 NeuronCores.

For each of 8 categorical columns: out = mu + softplus(rho) * eps gathered at
X[:, c]; outputs concatenated to [16384, 248] f32.

Strategy
  - Host packs each column's (mu, rho, eps) into one row-concatenated table so
    a single gathered row carries all three vectors.
  - Cols 0,1 (dim 64) -> group A, 256B rows [mu bf16 | rho+6 fp8e4m3 | eps
    fp8e4m3], vocab-sharded per column across the 8 cores; the host routes
    every (batch, col) pair to its owning core.  softplus uses Exp(x + -6.0)
    so the fp8 rho offset cancels inside the activation (f32 internally).
  - Cols 2,3 (dim 32) -> group B, mixed rows [mu f32 | rho bf16 | eps bf16]
    (256B rows), sharded + routed the same way.
  - Cols 4..6 (dim 16; f32 rows padded to 256B) -> group CS, table replicated,
    batch-sharded (core k handles batch rows [2048k, 2048k+2048)).
  - Col 7 (101 rows, dim 8): no gathers -- the 101-row table is DMA'd whole,
    softplus'd on-chip, and expanded with 16 PE one-hot matmuls
    (out[128b, 8] = onehot[101, 128b].T @ w7[101, 8], onehot sent from host).
  - Device per core: GPSIMD dma_gather (int16 indices; group A's rows are
    laid out in 5 32768-row sub-ranges with c0/c1 chunks interleaved so every
    sub-range gets ~1/5 of the lookups and indices fit int16), softplus via
    Exp + Ln(x+1) on ACT, mult/add on DVE (bf16), bf16 outputs stored
    per-segment so stores overlap later gathers.
  - Host scatters the routed rows back into the full output.

dma_gather contracts used here (see concourse/bass.py and bass_interp.py):
  - indices int16, element i at [i % 16, i // 16] of a [128, n/16] SBUF tile,
    that 16-row block replicated 8x down the partitions (one per Q7 core);
  - gathered row i lands at partition i % 128, slot i // 128 of the dst tile;
  - elem_size bytes must be a multiple of 256;
  - we pad every index segment with row 0 (always valid) so num_idxs is the
    same on all 8 cores (SPMD) and no -1 handling is needed.

Perf notes vs the f32 baseline (74.8us):
  - gathers are the bottleneck: each queue ring holds ~1024 descriptor pairs
    (CHUNK=1024 exactly), per-queue cycle ~= transfer + ~5us latency; bigger
    chunks/scratch/prepare_only all measured slower.
  - indices are sorted ascending per segment for HBM row locality (host
    unscrambles on assembly).
  - num_idxs registers hoisted; mlp library load issued first; a 128-idx
    warmup gather absorbs the first-call cost.
"""

import numpy as np

N_CORES = 8
BATCH = 16384
BPC = BATCH // N_CORES  # 2048 batch rows per core

VOCABS = [1000000, 200000, 100000, 50000, 10000, 5000, 1000, 100]
NROWS = [v + 1 for v in VOCABS]
DIMS = [64, 64, 32, 32, 16, 16, 16, 8]
OFFS = [0, 64, 128, 160, 192, 208, 224, 240]
DTOT = 248

A_COLS, B_COLS = (0, 1), (2, 3)
CS_GCOLS = (4, 5, 6)  # gathered; col 7 goes through the PE one-hot path
A_SH = [-(-NROWS[c] // N_CORES) for c in A_COLS]   # [125001, 25001]
A_W = 64                                           # 256B rows (bf16+fp8 pack)
SUB = 32768                                        # int16 sub-range size
N_RANGES = 5
# c0/c1 shard rows are split into N_RANGES chunks and interleaved per
# sub-range so each sub-range sees ~1/5 of both columns' lookups
def _split(n, k):
    q, r = divmod(n, k)
    return [q + (1 if i < r else 0) for i in range(k)]
A_P0 = _split(A_SH[0], N_RANGES)                   # c0 rows per sub-range
A_P1 = _split(A_SH[1], N_RANGES)                   # c1 rows per sub-range
A_START0 = np.cumsum([0] + A_P0)
A_START1 = np.cumsum([0] + A_P1)
assert all(p0 + p1 <= SUB for p0, p1 in zip(A_P0, A_P1))
S_A = (N_RANGES - 1) * SUB + A_P0[-1] + A_P1[-1]   # table rows incl. padding

B_SH = [-(-NROWS[c] // N_CORES) for c in B_COLS]   # [12501, 6251]
S_B = sum(B_SH)                                    # 18752
B_W = 64                                           # 256B rows (mixed pack)
CS_BASE = [0]
for c in (4, 5, 6):
    CS_BASE.append(CS_BASE[-1] + NROWS[c])
CS_ROWS = CS_BASE[-1] + NROWS[7]                   # 16104 (c7 rows at end)
C7_OFF = CS_BASE[-1]                               # first c7 row in TCS
CS_W = 64                                          # 256B rows
CS_N = BPC * len(CS_GCOLS)                         # 6144 gathered rows/core
CHUNK = 1024                                       # idx per dma_gather: one
                                                   # SWDGE queue ring holds
                                                   # ~1024 desc-pairs; bigger
                                                   # chunks stall Pool (80us),
                                                   # smaller waste cycles
WARMUP = True


def _chunks(cap):
    """Split cap into <=CHUNK pieces of near-equal 128-multiple size."""
    n = -(-cap // CHUNK)
    per = -(-cap // (n * 128)) * 128
    out = []
    c0 = 0
    while c0 < cap:
        c1 = min(c0 + per, cap)
        out.append((c0, c1))
        c0 = c1
    return out

_nc_cache = {}
last_result = None
RUN_MODE = "hw"  # "sim" runs CoreSim per core instead of hardware (debug)


def _build_nc(capsA, capB, softplus_native=True):
    """Build the SPMD Bacc program. capsA: rows gathered per A sub-range
    (each a multiple of 128, uniform across cores); capB likewise."""
    import concourse.bacc as bacc
    import concourse.bass as bass
    import concourse.mybir as mybir
    import concourse.tile as tile
    from concourse import library_config

    f32, i16 = mybir.dt.float32, mybir.dt.int16
    bf16, f8 = mybir.dt.bfloat16, mybir.dt.float8e4
    ACT = mybir.ActivationFunctionType
    ALU = mybir.AluOpType

    # Force Exp AND Ln onto the one ACT table containing both
    # (natural_log_exp_and_others): the table chooser otherwise alternates
    # exp_and_others <-> natural_log, reloading the table (1.28us) around
    # every chunk.  Table ids are dict positions, so only the function sets
    # are edited, never the order.
    if not getattr(bacc, "_ant_act_tables_patched", False):
        _orig_tables = bacc.get_activation_tables

        def _patched_tables(arch):
            t = dict(_orig_tables(arch))
            both = {mybir.ActivationFunctionType.Exp,
                    mybir.ActivationFunctionType.Ln}
            return {name: (fns if name == "natural_log_exp_and_others"
                           else fns - both)
                    for name, fns in t.items()}

        bacc.get_activation_tables = _patched_tables
        bacc._ant_act_tables_patched = True

    n_queues = 1 if not softplus_native else 4  # sim models 1 queue only
    nc = bacc.Bacc("TRN2", target_bir_lowering=False, debug=False,
                   num_swdge_queues=n_queues)

    TA = nc.dram_tensor("TA", [S_A, A_W], f32, kind="ExternalInput")
    TB = nc.dram_tensor("TB", [S_B, B_W], f32, kind="ExternalInput")
    TCS = nc.dram_tensor("TCS", [CS_ROWS, CS_W], f32, kind="ExternalInput")
    OH7 = nc.dram_tensor("OH7", [101, BPC], bf16, kind="ExternalInput")
    nI = sum(capsA) + capB + CS_N
    n0 = min(capsA[0], CHUNK)  # first gather's indices ride a tiny first DMA
    IDX0 = nc.dram_tensor("IDX0", [128, n0 // 16], i16, kind="ExternalInput")
    IDXR = nc.dram_tensor("IDXR", [128, (nI - n0) // 16], i16,
                          kind="ExternalInput")
    mA, mB = sum(capsA) // 128, capB // 128
    OA = nc.dram_tensor("OA", [128, mA * 64], bf16, kind="ExternalOutput")
    OB = nc.dram_tensor("OB", [128, mB * 32], bf16, kind="ExternalOutput")
    OC = nc.dram_tensor("OC", [128, 48 * 16], bf16, kind="ExternalOutput")
    OS = nc.dram_tensor("OS", [128, 16 * 8], bf16, kind="ExternalOutput")

    # gather segments, each <= CHUNK indices:
    # (name, src range, idx col offset, chunk cap, row width, dst slot base)
    segs = []
    o16 = 0
    slotA = 0
    for s in range(N_RANGES):
        r0, r1 = s * SUB, min(s * SUB + SUB, S_A)
        for c0, c1 in _chunks(capsA[s]):
            segs.append(("A", (r0, r1), o16, c1 - c0, A_W, slotA))
            o16 += (c1 - c0) // 16
            slotA += (c1 - c0) // 128
    slotB = 0
    for c0, c1 in _chunks(capB):
        segs.append(("B", (0, S_B), o16, c1 - c0, B_W, slotB))
        o16 += (c1 - c0) // 16
        slotB += (c1 - c0) // 128
    for c0, c1 in _chunks(CS_N):
        segs.append(("CS", (0, CS_ROWS), o16, c1 - c0, CS_W, c0 // 128))
        o16 += (c1 - c0) // 16

    # issue order: canonical seg 0 first (its indices ride the tiny first
    # DMA), then largest-first so the smallest segment's transfer is the tail
    order = [0] + sorted(range(1, len(segs)),
                         key=lambda i: (-segs[i][3] * segs[i][4], i))

    # greedy byte-balance across the SWDGE queues, in issue order
    qbytes = [0] * n_queues
    seg_queue = [0] * len(segs)
    for si in order:
        (name, _, _, cap, w, _) = segs[si]
        q = min(range(n_queues), key=lambda i: qbytes[i])
        qbytes[q] += cap * w * 4
        seg_queue[si] = q

    # start the ~13us GPSIMD library IRAM load (dma_gather lives in `mlp`)
    # as early as possible -- before the TileContext preamble
    nc.gpsimd.load_library(library_config.mlp)

    with tile.TileContext(nc) as tc:
        with tc.tile_pool(name="idx", bufs=1) as ipool, \
             tc.tile_pool(name="out", bufs=1) as opool, \
             tc.tile_pool(name="work", bufs=8) as wpool, \
             tc.tile_pool(name="ps", bufs=2,
                          space=bass.MemorySpace.PSUM) as ppool:

            # const for Exp(x - 6) (only 0.0/1.0 are pre-registered)
            cneg6 = ipool.tile([128, 1], f32, tag="c6")
            nc.vector.memset(cneg6[:], -6.0)
            nc.const_aps.aps[(f32, -6.0)] = cneg6[:]

            iz = ipool.tile([128, 8], i16, tag="iz")
            nc.gpsimd.memset(iz[:], 0)
            if WARMUP:
                # tiny gathers from a zeroed index tile, one per queue:
                # absorb the GPSIMD first-call + per-queue first-use cost
                # (~7us slow descgen observed on each queue's first real
                # gather) before IDX arrives; they drain concurrently.
                wz = ipool.tile([128, n_queues, CS_W], f32, tag="wz")
                r128 = nc.gpsimd.to_reg(128)
                for q in range(n_queues):
                    nc.gpsimd.dma_gather(
                        wz[:, q:q + 1, :], TCS.ap(), iz[:], 128,
                        r128, CS_W, queue_num=q,
                        single_packet=False)

            it0 = ipool.tile([128, n0 // 16], i16, tag="idx0")
            nc.sync.dma_start(it0[:], IDX0.ap())
            it = ipool.tile([128, (nI - n0) // 16], i16, tag="idx")
            nc.sync.dma_start(it[:], IDXR.ap())
            OAt = opool.tile([128, mA * 64], bf16, tag="OAt")
            OBt = opool.tile([128, mB * 32], bf16, tag="OBt")
            OCt = opool.tile([128, 48, 16], bf16, tag="OCt")
            OSt = opool.tile([128, 16, 8], bf16, tag="OSt")

            # one num_idxs register per distinct cap (the per-gather MOVE
            # otherwise delays dispatch by ~400ns each on Pool)
            regs = {}
            for (_, _, _, cap, _, _) in segs:
                if cap not in regs:
                    regs[cap] = nc.gpsimd.to_reg(cap)

            def emit_c7():
                # col 7: whole-table softplus + PE one-hot expansion.  Issued
                # mid-phase on the scalar HWDGE queue: early placement slows
                # the startup library-IRAM load (~4us), the sync queue would
                # serialize it behind the per-seg stores.
                oh = ipool.tile([101, BPC], bf16, tag="oh")
                nc.scalar.dma_start(oh[:], OH7.ap())
                w7g = ipool.tile([101, CS_W], f32, tag="w7g")
                nc.scalar.dma_start(w7g[:], TCS.ap()[C7_OFF:C7_OFF + 101, :])
                mu7, rho7 = w7g[:, 0:8], w7g[:, 8:16]
                eps7 = w7g[:, 16:24]
                nc.scalar.activation(rho7, rho7, ACT.Exp)
                nc.scalar.activation(rho7, rho7, ACT.Ln, bias=1.0)
                nc.vector.tensor_tensor(out=rho7, in0=rho7, in1=eps7,
                                        op=ALU.mult)
                w7 = ipool.tile([101, 8], bf16, tag="w7")
                nc.vector.tensor_tensor(out=w7[:], in0=rho7, in1=mu7,
                                        op=ALU.add)
                for t in range(16):
                    ps = ppool.tile([128, 8], f32, tag="ps", name=f"ps{t}")
                    nc.tensor.matmul(ps[:], oh[:, t * 128:(t + 1) * 128],
                                     w7[:])
                    nc.vector.tensor_copy(OSt[:, t, :], ps[:])
                nc.scalar.dma_start(
                    OS.ap(), OSt[:].rearrange("p a b -> p (a b)"))

            for pos, si in enumerate(order):
                (name, (r0, r1), off16, cap, w, slot0) = segs[si]
                if pos == 4:
                    emit_c7()
                mc = cap // 128
                src = (TA if name == "A" else TB if name == "B" else TCS)
                g = wpool.tile([128, mc, w], f32, tag=f"g{name}",
                               name=f"g{name}{si}")
                if si == 0:
                    idx_ap = it0[:, 0:cap // 16]
                else:
                    idx_ap = it[:, off16 - n0 // 16:
                                off16 - n0 // 16 + cap // 16]
                nc.gpsimd.dma_gather(
                    g[:], src.ap()[r0:r1, :], idx_ap,
                    cap, regs[cap], w, queue_num=seg_queue[si],
                    single_packet=False)
                if name == "A":
                    # rows [mu bf16 64 | rho+6 fp8 64 | eps fp8 64]
                    mu = g[:, :, 0:32].bitcast(bf16)
                    rho = g[:, :, 32:48].bitcast(f8)
                    eps = g[:, :, 48:64].bitcast(f8)
                    sp = wpool.tile([128, mc, 64], bf16, tag="spA",
                                    name=f"spA{si}")
                    out_ap = OAt[:, slot0 * 64:(slot0 + mc) * 64].rearrange(
                        "p (m d) -> p m d", d=64)
                    nc.scalar.activation(sp[:], rho, ACT.Exp, bias=-6.0)
                    nc.scalar.activation(sp[:], sp[:], ACT.Ln, bias=1.0)
                    nc.vector.tensor_tensor(out=sp[:], in0=sp[:], in1=eps,
                                            op=ALU.mult)
                    nc.vector.tensor_tensor(out=out_ap, in0=sp[:], in1=mu,
                                            op=ALU.add)
                    nc.sync.dma_start(
                        OA.ap()[:, slot0 * 64:(slot0 + mc) * 64],
                        OAt[:, slot0 * 64:(slot0 + mc) * 64])
                    continue
                if name == "B":
                    # rows [mu f32 32 | rho bf16 32 | eps bf16 32]
                    mu = g[:, :, 0:32]
                    rho = g[:, :, 32:48].bitcast(bf16)
                    eps = g[:, :, 48:64].bitcast(bf16)
                    out_ap = OBt[:, slot0 * 32:(slot0 + mc) * 32].rearrange(
                        "p (m d) -> p m d", d=32)
                    dst = OB.ap()[:, slot0 * 32:(slot0 + mc) * 32]
                    srcs = OBt[:, slot0 * 32:(slot0 + mc) * 32]
                else:
                    # f32 rows [mu | rho | eps | pad], d=16 (c4..c6)
                    # slots: i = c*2048 + sorted pos; slot-col j = c*16 + t
                    d = 16
                    mu = g[:, :, 0:d]
                    rho = g[:, :, d:2 * d]
                    eps = g[:, :, 2 * d:3 * d]
                    out_ap = OCt[:, slot0:slot0 + mc, :]
                    dst = OC.ap()[:, slot0 * 16:(slot0 + mc) * 16]
                    srcs = OCt[:, slot0:slot0 + mc, :].rearrange(
                        "p a b -> p (a b)")
                # softplus(rho) in place (Exp and Ln share one ACT table so
                # it stays resident; bias add happens in f32 inside ACT)
                nc.scalar.activation(rho, rho, ACT.Exp)
                nc.scalar.activation(rho, rho, ACT.Ln, bias=1.0)
                nc.vector.tensor_tensor(out=rho, in0=rho, in1=eps,
                                        op=ALU.mult)
                nc.vector.tensor_tensor(out=out_ap, in0=rho, in1=mu,
                                        op=ALU.add)
                nc.sync.dma_start(dst, srcs)
    nc.compile()
    return nc


def _pack3(mu, rho, eps, w):
    """Rows [mu | rho | eps | pad] of width w (f32)."""
    n, d = mu.shape
    out = np.zeros((n, w), dtype=np.float32)
    out[:, 0:d] = mu
    out[:, d:2 * d] = rho
    out[:, 2 * d:3 * d] = eps
    return out


def _pack3_fp8(mu, rho, eps, w):
    """Rows [mu bf16 d | rho+6 fp8e4m3 d | eps fp8e4m3 d], f32 width w=d."""
    import ml_dtypes
    n, d = mu.shape
    assert w == d
    buf = np.zeros((n, 4 * d), dtype=np.uint8)
    buf[:, 0:2 * d] = np.ascontiguousarray(
        mu.astype(ml_dtypes.bfloat16)).view(np.uint8).reshape(n, 2 * d)
    buf[:, 2 * d:3 * d] = np.ascontiguousarray(
        (rho + 6.0).astype(ml_dtypes.float8_e4m3fn)).view(np.uint8)
    buf[:, 3 * d:4 * d] = np.ascontiguousarray(
        eps.astype(ml_dtypes.float8_e4m3fn)).view(np.uint8)
    return buf.view(np.float32)


def _pack3_mixed(mu, rho, eps, w):
    """Rows [mu f32 d | rho bf16 d | eps bf16 d], f32 width w = 2d."""
    import ml_dtypes
    n, d = mu.shape
    assert w == 2 * d
    buf = np.empty((n, 4 * d), dtype=np.uint16)
    buf[:, 0:2 * d] = np.ascontiguousarray(mu).view(np.uint16)
    buf[:, 2 * d:3 * d] = np.ascontiguousarray(
        rho.astype(ml_dtypes.bfloat16)).view(np.uint16)
    buf[:, 3 * d:4 * d] = np.ascontiguousarray(
        eps.astype(ml_dtypes.bfloat16)).view(np.uint16)
    return buf.view(np.float32)


def _wrap16(arr):
    """int16 index array -> [128, n/16] dma_gather layout (i at [i%16, i//16],
    replicated 8x down the partition dim)."""
    n = len(arr)
    assert n % 16 == 0
    blk = arr.reshape(n // 16, 16).T  # [16, n/16]
    return np.tile(blk, (8, 1))


def _route(X, cols, shards):
    """Route (batch, col) pairs to per-column vocab-shard owners.

    Returns per-core (local_row, col_j, dest_b) arrays (unsorted)."""
    gid, owner, b_all, j_all = [], [], [], []
    for j, c in enumerate(cols):
        g = X[:, c].astype(np.int64)
        owner.append(g // shards[j])
        gid.append(g % shards[j])
        b_all.append(np.arange(BATCH, dtype=np.int64))
        j_all.append(np.full(BATCH, j, dtype=np.int64))
    gid = np.concatenate(gid)
    owner = np.concatenate(owner)
    b_all = np.concatenate(b_all)
    j_all = np.concatenate(j_all)
    order = np.argsort(owner, kind="stable")
    counts = np.bincount(owner, minlength=N_CORES)
    out = []
    start = 0
    for k in range(N_CORES):
        n = int(counts[k])
        sel = order[start:start + n]
        start += n
        out.append((gid[sel], j_all[sel], b_all[sel]))
    return out


def kernel(**inputs):
    from concourse.bass_utils import run_bass_kernel_spmd
    import ml_dtypes

    X = np.asarray(inputs["X"])
    mus = [np.asarray(inputs[f"mu{i}"], dtype=np.float32) for i in range(8)]
    rhos = [np.asarray(inputs[f"rho{i}"], dtype=np.float32) for i in range(8)]
    epss = [np.asarray(inputs[f"eps{i}"], dtype=np.float32) for i in range(8)]

    # ---- pack tables -----------------------------------------------------
    # A: per-core table of N_RANGES sub-ranges, each holding a chunk of the
    # c0 shard followed by a chunk of the c1 shard (interleaved for balance)
    packedA = [_pack3_fp8(mus[c], rhos[c], epss[c], A_W) for c in A_COLS]
    WA = []
    for k in range(N_CORES):
        tbl = np.zeros((S_A, A_W), dtype=np.float32)
        for s in range(N_RANGES):
            base = s * SUB
            for j, (p, st) in enumerate(((A_P0, A_START0), (A_P1, A_START1))):
                # shard k's rows run [k*A_SH[j], min((k+1)*A_SH[j], NROWS));
                # the last core's shard is short -- clip and zero-pad
                lo = k * A_SH[j] + st[s]
                hi = min(lo + p[s], NROWS[A_COLS[j]])
                if lo >= hi:
                    continue
                off = base + (A_P0[s] if j == 1 else 0)
                tbl[off:off + hi - lo] = packedA[j][lo:hi]
        WA.append(tbl)

    def shard_tables(cols, shards, w, pack):
        packed = [pack(mus[c], rhos[c], epss[c], w) for c in cols]
        per_core = []
        for k in range(N_CORES):
            parts = []
            for j, p in enumerate(packed):
                sh = np.zeros((shards[j], w), dtype=np.float32)
                src = p[k * shards[j]:(k + 1) * shards[j]]
                sh[:len(src)] = src
                parts.append(sh)
            per_core.append(np.concatenate(parts))
        return per_core

    WB = shard_tables(B_COLS, B_SH, B_W, _pack3_mixed)
    WCS = np.concatenate(
        [_pack3(mus[c], rhos[c], epss[c], CS_W) for c in (4, 5, 6, 7)])

    # ---- route A and B ---------------------------------------------------
    routedA = _route(X, A_COLS, A_SH)
    routedB = _route(X, B_COLS, B_SH)

    # A: map local rows to (sub-range, in-range idx) via the interleaved
    # layout, bucket by sub-range, sort each bucket ascending
    bucketsA = []  # [core][s] -> (idx16, dest_b, dest_j)
    for k in range(N_CORES):
        loc, jj, bb = routedA[k]
        s_of = np.empty(len(loc), dtype=np.int64)
        i16 = np.empty(len(loc), dtype=np.int64)
        for j, st in ((0, A_START0), (1, A_START1)):
            sel = jj == j
            s = np.searchsorted(st[1:], loc[sel], side="right")
            base = loc[sel] - st[s]
            if j == 1:
                base = base + np.asarray(A_P0)[s]
            s_of[sel] = s
            i16[sel] = base
        per = []
        for s in range(N_RANGES):
            sel = s_of == s
            v, bs, js = i16[sel], bb[sel], jj[sel]
            o = np.argsort(v, kind="stable")
            per.append((v[o].astype(np.int16), bs[o], js[o]))
        bucketsA.append(per)
    capsA = [max(128, -(-max(len(bucketsA[k][s][0]) for k in range(N_CORES))
                        // 128) * 128) for s in range(N_RANGES)]

    # B: stacked-shard local rows, sorted ascending
    locsB, destB = [], []
    col_offB = np.cumsum([0] + list(B_SH[:-1]))
    for k in range(N_CORES):
        loc, jj, bb = routedB[k]
        loc = loc + col_offB[jj]
        o = np.argsort(loc, kind="stable")
        locsB.append(loc[o])
        destB.append((bb[o], jj[o]))
    capB = max(128, -(-max(len(locsB[k]) for k in range(N_CORES)) // 128) * 128)

    key = (tuple(capsA), capB, RUN_MODE)
    if key not in _nc_cache:
        _nc_cache[key] = _build_nc(list(capsA), capB,
                                   softplus_native=(RUN_MODE != "sim"))
    nc = _nc_cache[key]

    # ---- per-core inputs -------------------------------------------------
    in_maps = []
    permCS = [[] for _ in range(N_CORES)]  # [core][j] -> batch perm of col j
    for k in range(N_CORES):
        segs16 = []

        def add_wrapped(arr, cap):
            full = np.zeros(cap, dtype=np.int16)
            full[:len(arr)] = arr
            for c0, c1 in _chunks(cap):
                segs16.append(_wrap16(full[c0:c1]))

        for s in range(N_RANGES):
            add_wrapped(bucketsA[k][s][0], capsA[s])
        add_wrapped(locsB[k].astype(np.int16), capB)
        Xk = X[k * BPC:(k + 1) * BPC]
        cols_sorted = []
        for j, c in enumerate(CS_GCOLS):
            o = np.argsort(Xk[:, c], kind="stable")
            permCS[k].append(o)
            cols_sorted.append(Xk[o, c].astype(np.int16) + CS_BASE[j])
        add_wrapped(np.concatenate(cols_sorted), CS_N)
        oh = np.zeros((101, BPC), dtype=ml_dtypes.bfloat16)
        oh[Xk[:, 7].astype(np.int64), np.arange(BPC)] = 1.0
        in_maps.append({
            "TA": WA[k],
            "TB": WB[k],
            "TCS": WCS,
            "OH7": oh,
            "IDX0": np.ascontiguousarray(segs16[0]),
            "IDXR": np.ascontiguousarray(np.concatenate(segs16[1:], axis=1)),
        })

    global last_result
    if RUN_MODE == "sim":
        from concourse.bass_interp import CoreSim
        results = []
        for im in in_maps:
            sim = CoreSim(nc, trace=False)
            for kk, v in im.items():
                sim.tensor(kk)[:] = v
            sim.simulate()
            results.append({o: np.array(sim.mem_tensor(o))
                            for o in ("OA", "OB", "OC", "OS")})
        last_result = None
    else:
        res = run_bass_kernel_spmd(nc, in_maps, core_ids=list(range(N_CORES)))
        last_result = res
        results = res.results

    # ---- assemble output -------------------------------------------------
    OUT = np.empty((BATCH, DTOT), dtype=np.float32)

    def unslot(seg, cap, d):
        # device slot i -> [i % 128, i // 128]; seg is [128, (cap//128)*d]
        seg = np.asarray(seg).astype(np.float32)
        return seg.reshape(128, cap // 128, d).transpose(1, 0, 2).reshape(cap, d)

    for k in range(N_CORES):
        oa = np.asarray(results[k]["OA"])
        a_off = 0
        for s in range(N_RANGES):
            mc = capsA[s] // 128
            rows = unslot(oa[:, a_off * 64:(a_off + mc) * 64], capsA[s], 64)
            a_off += mc
            _, b, j = bucketsA[k][s]
            n = len(b)
            for jj, col in enumerate(A_COLS):
                sel = j == jj
                OUT[b[sel], OFFS[col]:OFFS[col] + 64] = rows[:n][sel]
        rowsB = unslot(results[k]["OB"], capB, 32)
        b, j = destB[k]
        n = len(b)
        for jj, col in enumerate(B_COLS):
            sel = j == jj
            OUT[b[sel], OFFS[col]:OFFS[col] + 32] = rowsB[:n][sel]
        # OC: [128, c(3), t(16), 16] with slot-col j = c*16 + t; slot order is
        # the per-column sorted order, so scatter back through permCS
        oc = np.asarray(results[k]["OC"]).astype(np.float32)
        oc = oc.reshape(128, 3, 16, 16)
        for j, col in enumerate(CS_GCOLS):
            blk = oc[:, j].transpose(1, 0, 2).reshape(BPC, 16)
            OUT[k * BPC + permCS[k][j], OFFS[col]:OFFS[col] + 16] = blk
        # OS: matmul tile t partition p -> batch row t*128+p (identity)
        os_ = np.asarray(results[k]["OS"]).astype(np.float32)
        os_ = os_.reshape(128, 16, 8).transpose(1, 0, 2).reshape(BPC, 8)
        OUT[k * BPC:(k + 1) * BPC, OFFS[7]:OFFS[7] + 8] = os_
    return OUT


# revision 39
# speedup vs baseline: 1.2179x; 1.2179x over previous
"""Bayesian categorical embedding lookup on 8 trn2 NeuronCores.

For each of 8 categorical columns: out = mu + softplus(rho) * eps gathered at
X[:, c]; outputs concatenated to [16384, 248] f32.

Strategy
  - Host packs each column's (mu, rho, eps) into one row-concatenated table so
    a single gathered row carries all three vectors.
  - Cols 0,1 (dim 64) -> group A, 256B rows [mu bf16 | rho+6 fp8e4m3 | eps
    fp8e4m3], vocab-sharded per column across the 8 cores; the host routes
    every (batch, col) pair to its owning core.  softplus uses Exp(x + -6.0)
    so the fp8 rho offset cancels inside the activation (f32 internally).
  - Cols 2,3 (dim 32) -> group B, mixed rows [mu f32 | rho bf16 | eps bf16]
    (256B rows), sharded + routed the same way.
  - Cols 4..6 (dim 16; f32 rows padded to 256B) -> group CS, table replicated,
    batch-sharded (core k handles batch rows [2048k, 2048k+2048)).
  - Col 7 (101 rows, dim 8): no gathers -- the 101-row table is DMA'd whole,
    softplus'd on-chip, and expanded with 16 PE one-hot matmuls
    (out[128b, 8] = onehot[101, 128b].T @ w7[101, 8], onehot sent from host).
  - Device per core: GPSIMD dma_gather (int16 indices; group A's rows are
    laid out in 5 32768-row sub-ranges with c0/c1 chunks interleaved so every
    sub-range gets ~1/5 of the lookups and indices fit int16), softplus via
    Exp + Ln(x+1) on ACT, mult/add on DVE (bf16), bf16 outputs stored
    per-segment so stores overlap later gathers.
  - Host scatters the routed rows back into the full output.

dma_gather contracts used here (see concourse/bass.py and bass_interp.py):
  - indices int16, element i at [i % 16, i // 16] of a [128, n/16] SBUF tile,
    that 16-row block replicated 8x down the partitions (one per Q7 core);
  - gathered row i lands at partition i % 128, slot i // 128 of the dst tile;
  - elem_size bytes must be a multiple of 256;
  - we pad every index segment with row 0 (always valid) so num_idxs is the
    same on all 8 cores (SPMD) and no -1 handling is needed.

Perf notes vs the f32 baseline (74.8us):
  - gathers are the bottleneck: each queue ring holds ~1024 descriptor pairs
    (CHUNK=1024 exactly), per-queue cycle ~= transfer + ~5us latency; bigger
    chunks/scratch/prepare_only all measured slower.
  - indices are sorted ascending per segment for HBM row locality (host
    unscrambles on assembly).
  - num_idxs registers hoisted; mlp library load issued first; a 128-idx
    warmup gather absorbs the first-call cost.
"""

import numpy as np

N_CORES = 8
BATCH = 16384
BPC = BATCH // N_CORES  # 2048 batch rows per core

VOCABS = [1000000, 200000, 100000, 50000, 10000, 5000, 1000, 100]
NROWS = [v + 1 for v in VOCABS]
DIMS = [64, 64, 32, 32, 16, 16, 16, 8]
OFFS = [0, 64, 128, 160, 192, 208, 224, 240]
DTOT = 248

A_COLS, B_COLS = (0, 1), (2, 3)
CS_GCOLS = (4, 5, 6)  # gathered; col 7 goes through the PE one-hot path
A_SH = [-(-NROWS[c] // N_CORES) for c in A_COLS]   # [125001, 25001]
A_W = 64                                           # 256B rows (bf16+fp8 pack)
SUB = 32768                                        # int16 sub-range size
N_RANGES = 5
# c0/c1 shard rows are split into N_RANGES chunks and interleaved per
# sub-range so each sub-range sees ~1/5 of both columns' lookups
def _split(n, k):
    q, r = divmod(n, k)
    return [q + (1 if i < r else 0) for i in range(k)]
A_P0 = _split(A_SH[0], N_RANGES)                   # c0 rows per sub-range
A_P1 = _split(A_SH[1], N_RANGES)                   # c1 rows per sub-range
A_START0 = np.cumsum([0] + A_P0)
A_START1 = np.cumsum([0] + A_P1)
assert all(p0 + p1 <= SUB for p0, p1 in zip(A_P0, A_P1))
S_A = (N_RANGES - 1) * SUB + A_P0[-1] + A_P1[-1]   # table rows incl. padding

B_SH = [-(-NROWS[c] // N_CORES) for c in B_COLS]   # [12501, 6251]
S_B = sum(B_SH)                                    # 18752
B_W = 64                                           # 256B rows (mixed pack)
CS_BASE = [0]
for c in (4, 5, 6):
    CS_BASE.append(CS_BASE[-1] + NROWS[c])
CS_ROWS = CS_BASE[-1] + NROWS[7]                   # 16104 (c7 rows at end)
C7_OFF = CS_BASE[-1]                               # first c7 row in TCS
CS_W = 64                                          # 256B rows
CS_N = BPC * len(CS_GCOLS)                         # 6144 gathered rows/core
CHUNK = 1024                                       # idx per dma_gather: one
                                                   # SWDGE queue ring holds
                                                   # ~1024 desc-pairs; bigger
                                                   # chunks stall Pool (80us),
                                                   # smaller waste cycles
WARMUP = True


def _chunks(cap):
    """Split cap into <=CHUNK pieces of near-equal 128-multiple size."""
    n = -(-cap // CHUNK)
    per = -(-cap // (n * 128)) * 128
    out = []
    c0 = 0
    while c0 < cap:
        c1 = min(c0 + per, cap)
        out.append((c0, c1))
        c0 = c1
    return out

_nc_cache = {}
last_result = None
RUN_MODE = "hw"  # "sim" runs CoreSim per core instead of hardware (debug)


def _build_nc(capsA, capB, softplus_native=True):
    """Build the SPMD Bacc program. capsA: rows gathered per A sub-range
    (each a multiple of 128, uniform across cores); capB likewise."""
    import concourse.bacc as bacc
    import concourse.bass as bass
    import concourse.mybir as mybir
    import concourse.tile as tile
    from concourse import library_config

    f32, i16 = mybir.dt.float32, mybir.dt.int16
    bf16, f8 = mybir.dt.bfloat16, mybir.dt.float8e4
    ACT = mybir.ActivationFunctionType
    ALU = mybir.AluOpType

    # Force Exp AND Ln onto the one ACT table containing both
    # (natural_log_exp_and_others): the table chooser otherwise alternates
    # exp_and_others <-> natural_log, reloading the table (1.28us) around
    # every chunk.  Table ids are dict positions, so only the function sets
    # are edited, never the order.
    if not getattr(bacc, "_ant_act_tables_patched", False):
        _orig_tables = bacc.get_activation_tables

        def _patched_tables(arch):
            t = dict(_orig_tables(arch))
            both = {mybir.ActivationFunctionType.Exp,
                    mybir.ActivationFunctionType.Ln}
            return {name: (fns if name == "natural_log_exp_and_others"
                           else fns - both)
                    for name, fns in t.items()}

        bacc.get_activation_tables = _patched_tables
        bacc._ant_act_tables_patched = True

    n_queues = 1 if not softplus_native else 4  # sim models 1 queue only
    nc = bacc.Bacc("TRN2", target_bir_lowering=False, debug=False,
                   num_swdge_queues=n_queues)

    TA = nc.dram_tensor("TA", [S_A, A_W], f32, kind="ExternalInput")
    TB = nc.dram_tensor("TB", [S_B, B_W], f32, kind="ExternalInput")
    TCS = nc.dram_tensor("TCS", [CS_ROWS, CS_W], f32, kind="ExternalInput")
    OH7 = nc.dram_tensor("OH7", [101, BPC], bf16, kind="ExternalInput")
    nI = sum(capsA) + capB + CS_N
    n0 = min(capsA[0], CHUNK)  # first gather's indices ride a tiny first DMA
    IDX0 = nc.dram_tensor("IDX0", [128, n0 // 16], i16, kind="ExternalInput")
    IDXR = nc.dram_tensor("IDXR", [128, (nI - n0) // 16], i16,
                          kind="ExternalInput")
    mA, mB = sum(capsA) // 128, capB // 128
    OA = nc.dram_tensor("OA", [128, mA * 64], bf16, kind="ExternalOutput")
    OB = nc.dram_tensor("OB", [128, mB * 32], bf16, kind="ExternalOutput")
    OC = nc.dram_tensor("OC", [128, 48 * 16], bf16, kind="ExternalOutput")
    OS = nc.dram_tensor("OS", [128, 16 * 8], bf16, kind="ExternalOutput")

    # gather segments, each <= CHUNK indices:
    # (name, src range, idx col offset, chunk cap, row width, dst slot base)
    segs = []
    o16 = 0
    slotA = 0
    for s in range(N_RANGES):
        r0, r1 = s * SUB, min(s * SUB + SUB, S_A)
        for c0, c1 in _chunks(capsA[s]):
            segs.append(("A", (r0, r1), o16, c1 - c0, A_W, slotA))
            o16 += (c1 - c0) // 16
            slotA += (c1 - c0) // 128
    slotB = 0
    for c0, c1 in _chunks(capB):
        segs.append(("B", (0, S_B), o16, c1 - c0, B_W, slotB))
        o16 += (c1 - c0) // 16
        slotB += (c1 - c0) // 128
    for c0, c1 in _chunks(CS_N):
        segs.append(("CS", (0, CS_ROWS), o16, c1 - c0, CS_W, c0 // 128))
        o16 += (c1 - c0) // 16

    # issue order: canonical seg 0 first (its indices ride the tiny first
    # DMA), then largest-first so the smallest segment's transfer is the tail
    order = [0] + sorted(range(1, len(segs)),
                         key=lambda i: (-segs[i][3] * segs[i][4], i))

    # greedy byte-balance across the SWDGE queues, in issue order
    qbytes = [0] * n_queues
    seg_queue = [0] * len(segs)
    for si in order:
        (name, _, _, cap, w, _) = segs[si]
        q = min(range(n_queues), key=lambda i: qbytes[i])
        qbytes[q] += cap * w * 4
        seg_queue[si] = q

    # start the ~13us GPSIMD library IRAM load (dma_gather lives in `mlp`)
    # as early as possible -- before the TileContext preamble
    nc.gpsimd.load_library(library_config.mlp)

    with tile.TileContext(nc) as tc:
        with tc.tile_pool(name="idx", bufs=1) as ipool, \
             tc.tile_pool(name="out", bufs=1) as opool, \
             tc.tile_pool(name="work", bufs=8) as wpool, \
             tc.tile_pool(name="ps", bufs=2,
                          space=bass.MemorySpace.PSUM) as ppool:

            # const for Exp(x - 6) (only 0.0/1.0 are pre-registered)
            cneg6 = ipool.tile([128, 1], f32, tag="c6")
            nc.vector.memset(cneg6[:], -6.0)
            nc.const_aps.aps[(f32, -6.0)] = cneg6[:]

            iz = ipool.tile([128, 8], i16, tag="iz")
            nc.gpsimd.memset(iz[:], 0)
            if WARMUP:
                # tiny gather from a zeroed index tile: triggers the GPSIMD
                # first-call IRAM cost before IDX arrives.  (One warmup per
                # queue measured 80.7us vs 65.9us -- do NOT widen this.)
                wz = ipool.tile([128, 1, CS_W], f32, tag="wz")
                nc.gpsimd.dma_gather(
                    wz[:], TCS.ap(), iz[:], 128,
                    nc.gpsimd.to_reg(128), CS_W, queue_num=0,
                    single_packet=False)

            it0 = ipool.tile([128, n0 // 16], i16, tag="idx0")
            nc.sync.dma_start(it0[:], IDX0.ap())
            it = ipool.tile([128, (nI - n0) // 16], i16, tag="idx")
            nc.sync.dma_start(it[:], IDXR.ap())
            OAt = opool.tile([128, mA * 64], bf16, tag="OAt")
            OBt = opool.tile([128, mB * 32], bf16, tag="OBt")
            OCt = opool.tile([128, 48, 16], bf16, tag="OCt")
            OSt = opool.tile([128, 16, 8], bf16, tag="OSt")

            # one num_idxs register per distinct cap (the per-gather MOVE
            # otherwise delays dispatch by ~400ns each on Pool)
            regs = {}
            for (_, _, _, cap, _, _) in segs:
                if cap not in regs:
                    regs[cap] = nc.gpsimd.to_reg(cap)

            def emit_c7():
                # col 7: whole-table softplus + PE one-hot expansion.  Issued
                # mid-phase on the scalar HWDGE queue: early placement slows
                # the startup library-IRAM load (~4us), the sync queue would
                # serialize it behind the per-seg stores.
                oh = ipool.tile([101, BPC], bf16, tag="oh")
                nc.scalar.dma_start(oh[:], OH7.ap())
                w7g = ipool.tile([101, CS_W], f32, tag="w7g")
                nc.scalar.dma_start(w7g[:], TCS.ap()[C7_OFF:C7_OFF + 101, :])
                mu7, rho7 = w7g[:, 0:8], w7g[:, 8:16]
                eps7 = w7g[:, 16:24]
                nc.scalar.activation(rho7, rho7, ACT.Exp)
                nc.scalar.activation(rho7, rho7, ACT.Ln, bias=1.0)
                nc.vector.tensor_tensor(out=rho7, in0=rho7, in1=eps7,
                                        op=ALU.mult)
                w7 = ipool.tile([101, 8], bf16, tag="w7")
                nc.vector.tensor_tensor(out=w7[:], in0=rho7, in1=mu7,
                                        op=ALU.add)
                for t in range(16):
                    ps = ppool.tile([128, 8], f32, tag="ps", name=f"ps{t}")
                    nc.tensor.matmul(ps[:], oh[:, t * 128:(t + 1) * 128],
                                     w7[:])
                    nc.vector.tensor_copy(OSt[:, t, :], ps[:])
                nc.scalar.dma_start(
                    OS.ap(), OSt[:].rearrange("p a b -> p (a b)"))

            for pos, si in enumerate(order):
                (name, (r0, r1), off16, cap, w, slot0) = segs[si]
                if pos == 4:
                    emit_c7()
                mc = cap // 128
                src = (TA if name == "A" else TB if name == "B" else TCS)
                g = wpool.tile([128, mc, w], f32, tag=f"g{name}",
                               name=f"g{name}{si}")
                if si == 0:
                    idx_ap = it0[:, 0:cap // 16]
                else:
                    idx_ap = it[:, off16 - n0 // 16:
                                off16 - n0 // 16 + cap // 16]
                nc.gpsimd.dma_gather(
                    g[:], src.ap()[r0:r1, :], idx_ap,
                    cap, regs[cap], w, queue_num=seg_queue[si],
                    single_packet=False)
                if name == "A":
                    # rows [mu bf16 64 | rho+6 fp8 64 | eps fp8 64]
                    mu = g[:, :, 0:32].bitcast(bf16)
                    rho = g[:, :, 32:48].bitcast(f8)
                    eps = g[:, :, 48:64].bitcast(f8)
                    sp = wpool.tile([128, mc, 64], bf16, tag="spA",
                                    name=f"spA{si}")
                    out_ap = OAt[:, slot0 * 64:(slot0 + mc) * 64].rearrange(
                        "p (m d) -> p m d", d=64)
                    nc.scalar.activation(sp[:], rho, ACT.Exp, bias=-6.0)
                    nc.scalar.activation(sp[:], sp[:], ACT.Ln, bias=1.0)
                    nc.vector.tensor_tensor(out=sp[:], in0=sp[:], in1=eps,
                                            op=ALU.mult)
                    nc.vector.tensor_tensor(out=out_ap, in0=sp[:], in1=mu,
                                            op=ALU.add)
                    nc.sync.dma_start(
                        OA.ap()[:, slot0 * 64:(slot0 + mc) * 64],
                        OAt[:, slot0 * 64:(slot0 + mc) * 64])
                    continue
                if name == "B":
                    # rows [mu f32 32 | rho bf16 32 | eps bf16 32]
                    mu = g[:, :, 0:32]
                    rho = g[:, :, 32:48].bitcast(bf16)
                    eps = g[:, :, 48:64].bitcast(bf16)
                    out_ap = OBt[:, slot0 * 32:(slot0 + mc) * 32].rearrange(
                        "p (m d) -> p m d", d=32)
                    dst = OB.ap()[:, slot0 * 32:(slot0 + mc) * 32]
                    srcs = OBt[:, slot0 * 32:(slot0 + mc) * 32]
                else:
                    # f32 rows [mu | rho | eps | pad], d=16 (c4..c6)
                    # slots: i = c*2048 + sorted pos; slot-col j = c*16 + t
                    d = 16
                    mu = g[:, :, 0:d]
                    rho = g[:, :, d:2 * d]
                    eps = g[:, :, 2 * d:3 * d]
                    out_ap = OCt[:, slot0:slot0 + mc, :]
                    dst = OC.ap()[:, slot0 * 16:(slot0 + mc) * 16]
                    srcs = OCt[:, slot0:slot0 + mc, :].rearrange(
                        "p a b -> p (a b)")
                # softplus(rho) in place (Exp and Ln share one ACT table so
                # it stays resident; bias add happens in f32 inside ACT)
                nc.scalar.activation(rho, rho, ACT.Exp)
                nc.scalar.activation(rho, rho, ACT.Ln, bias=1.0)
                nc.vector.tensor_tensor(out=rho, in0=rho, in1=eps,
                                        op=ALU.mult)
                nc.vector.tensor_tensor(out=out_ap, in0=rho, in1=mu,
                                        op=ALU.add)
                nc.sync.dma_start(dst, srcs)
    nc.compile()
    return nc


def _pack3(mu, rho, eps, w):
    """Rows [mu | rho | eps | pad] of width w (f32)."""
    n, d = mu.shape
    out = np.zeros((n, w), dtype=np.float32)
    out[:, 0:d] = mu
    out[:, d:2 * d] = rho
    out[:, 2 * d:3 * d] = eps
    return out


def _pack3_fp8(mu, rho, eps, w):
    """Rows [mu bf16 d | rho+6 fp8e4m3 d | eps fp8e4m3 d], f32 width w=d."""
    import ml_dtypes
    n, d = mu.shape
    assert w == d
    buf = np.zeros((n, 4 * d), dtype=np.uint8)
    buf[:, 0:2 * d] = np.ascontiguousarray(
        mu.astype(ml_dtypes.bfloat16)).view(np.uint8).reshape(n, 2 * d)
    buf[:, 2 * d:3 * d] = np.ascontiguousarray(
        (rho + 6.0).astype(ml_dtypes.float8_e4m3fn)).view(np.uint8)
    buf[:, 3 * d:4 * d] = np.ascontiguousarray(
        eps.astype(ml_dtypes.float8_e4m3fn)).view(np.uint8)
    return buf.view(np.float32)


def _pack3_mixed(mu, rho, eps, w):
    """Rows [mu f32 d | rho bf16 d | eps bf16 d], f32 width w = 2d."""
    import ml_dtypes
    n, d = mu.shape
    assert w == 2 * d
    buf = np.empty((n, 4 * d), dtype=np.uint16)
    buf[:, 0:2 * d] = np.ascontiguousarray(mu).view(np.uint16)
    buf[:, 2 * d:3 * d] = np.ascontiguousarray(
        rho.astype(ml_dtypes.bfloat16)).view(np.uint16)
    buf[:, 3 * d:4 * d] = np.ascontiguousarray(
        eps.astype(ml_dtypes.bfloat16)).view(np.uint16)
    return buf.view(np.float32)


def _wrap16(arr):
    """int16 index array -> [128, n/16] dma_gather layout (i at [i%16, i//16],
    replicated 8x down the partition dim)."""
    n = len(arr)
    assert n % 16 == 0
    blk = arr.reshape(n // 16, 16).T  # [16, n/16]
    return np.tile(blk, (8, 1))


def _route(X, cols, shards):
    """Route (batch, col) pairs to per-column vocab-shard owners.

    Returns per-core (local_row, col_j, dest_b) arrays (unsorted)."""
    gid, owner, b_all, j_all = [], [], [], []
    for j, c in enumerate(cols):
        g = X[:, c].astype(np.int64)
        owner.append(g // shards[j])
        gid.append(g % shards[j])
        b_all.append(np.arange(BATCH, dtype=np.int64))
        j_all.append(np.full(BATCH, j, dtype=np.int64))
    gid = np.concatenate(gid)
    owner = np.concatenate(owner)
    b_all = np.concatenate(b_all)
    j_all = np.concatenate(j_all)
    order = np.argsort(owner, kind="stable")
    counts = np.bincount(owner, minlength=N_CORES)
    out = []
    start = 0
    for k in range(N_CORES):
        n = int(counts[k])
        sel = order[start:start + n]
        start += n
        out.append((gid[sel], j_all[sel], b_all[sel]))
    return out


def kernel(**inputs):
    from concourse.bass_utils import run_bass_kernel_spmd
    import ml_dtypes

    X = np.asarray(inputs["X"])
    mus = [np.asarray(inputs[f"mu{i}"], dtype=np.float32) for i in range(8)]
    rhos = [np.asarray(inputs[f"rho{i}"], dtype=np.float32) for i in range(8)]
    epss = [np.asarray(inputs[f"eps{i}"], dtype=np.float32) for i in range(8)]

    # ---- pack tables -----------------------------------------------------
    # A: per-core table of N_RANGES sub-ranges, each holding a chunk of the
    # c0 shard followed by a chunk of the c1 shard (interleaved for balance)
    packedA = [_pack3_fp8(mus[c], rhos[c], epss[c], A_W) for c in A_COLS]
    WA = []
    for k in range(N_CORES):
        tbl = np.zeros((S_A, A_W), dtype=np.float32)
        for s in range(N_RANGES):
            base = s * SUB
            for j, (p, st) in enumerate(((A_P0, A_START0), (A_P1, A_START1))):
                # shard k's rows run [k*A_SH[j], min((k+1)*A_SH[j], NROWS));
                # the last core's shard is short -- clip and zero-pad
                lo = k * A_SH[j] + st[s]
                hi = min(lo + p[s], NROWS[A_COLS[j]])
                if lo >= hi:
                    continue
                off = base + (A_P0[s] if j == 1 else 0)
                tbl[off:off + hi - lo] = packedA[j][lo:hi]
        WA.append(tbl)

    def shard_tables(cols, shards, w, pack):
        packed = [pack(mus[c], rhos[c], epss[c], w) for c in cols]
        per_core = []
        for k in range(N_CORES):
            parts = []
            for j, p in enumerate(packed):
                sh = np.zeros((shards[j], w), dtype=np.float32)
                src = p[k * shards[j]:(k + 1) * shards[j]]
                sh[:len(src)] = src
                parts.append(sh)
            per_core.append(np.concatenate(parts))
        return per_core

    WB = shard_tables(B_COLS, B_SH, B_W, _pack3_mixed)
    WCS = np.concatenate(
        [_pack3(mus[c], rhos[c], epss[c], CS_W) for c in (4, 5, 6, 7)])

    # ---- route A and B ---------------------------------------------------
    routedA = _route(X, A_COLS, A_SH)
    routedB = _route(X, B_COLS, B_SH)

    # A: map local rows to (sub-range, in-range idx) via the interleaved
    # layout, bucket by sub-range, sort each bucket ascending
    bucketsA = []  # [core][s] -> (idx16, dest_b, dest_j)
    for k in range(N_CORES):
        loc, jj, bb = routedA[k]
        s_of = np.empty(len(loc), dtype=np.int64)
        i16 = np.empty(len(loc), dtype=np.int64)
        for j, st in ((0, A_START0), (1, A_START1)):
            sel = jj == j
            s = np.searchsorted(st[1:], loc[sel], side="right")
            base = loc[sel] - st[s]
            if j == 1:
                base = base + np.asarray(A_P0)[s]
            s_of[sel] = s
            i16[sel] = base
        per = []
        for s in range(N_RANGES):
            sel = s_of == s
            v, bs, js = i16[sel], bb[sel], jj[sel]
            o = np.argsort(v, kind="stable")
            per.append((v[o].astype(np.int16), bs[o], js[o]))
        bucketsA.append(per)
    capsA = [max(128, -(-max(len(bucketsA[k][s][0]) for k in range(N_CORES))
                        // 128) * 128) for s in range(N_RANGES)]

    # B: stacked-shard local rows, sorted ascending
    locsB, destB = [], []
    col_offB = np.cumsum([0] + list(B_SH[:-1]))
    for k in range(N_CORES):
        loc, jj, bb = routedB[k]
        loc = loc + col_offB[jj]
        o = np.argsort(loc, kind="stable")
        locsB.append(loc[o])
        destB.append((bb[o], jj[o]))
    capB = max(128, -(-max(len(locsB[k]) for k in range(N_CORES)) // 128) * 128)

    key = (tuple(capsA), capB, RUN_MODE)
    if key not in _nc_cache:
        _nc_cache[key] = _build_nc(list(capsA), capB,
                                   softplus_native=(RUN_MODE != "sim"))
    nc = _nc_cache[key]

    # ---- per-core inputs -------------------------------------------------
    in_maps = []
    permCS = [[] for _ in range(N_CORES)]  # [core][j] -> batch perm of col j
    for k in range(N_CORES):
        segs16 = []

        def add_wrapped(arr, cap):
            full = np.zeros(cap, dtype=np.int16)
            full[:len(arr)] = arr
            for c0, c1 in _chunks(cap):
                segs16.append(_wrap16(full[c0:c1]))

        for s in range(N_RANGES):
            add_wrapped(bucketsA[k][s][0], capsA[s])
        add_wrapped(locsB[k].astype(np.int16), capB)
        Xk = X[k * BPC:(k + 1) * BPC]
        cols_sorted = []
        for j, c in enumerate(CS_GCOLS):
            o = np.argsort(Xk[:, c], kind="stable")
            permCS[k].append(o)
            cols_sorted.append(Xk[o, c].astype(np.int16) + CS_BASE[j])
        add_wrapped(np.concatenate(cols_sorted), CS_N)
        oh = np.zeros((101, BPC), dtype=ml_dtypes.bfloat16)
        oh[Xk[:, 7].astype(np.int64), np.arange(BPC)] = 1.0
        in_maps.append({
            "TA": WA[k],
            "TB": WB[k],
            "TCS": WCS,
            "OH7": oh,
            "IDX0": np.ascontiguousarray(segs16[0]),
            "IDXR": np.ascontiguousarray(np.concatenate(segs16[1:], axis=1)),
        })

    global last_result
    if RUN_MODE == "sim":
        from concourse.bass_interp import CoreSim
        results = []
        for im in in_maps:
            sim = CoreSim(nc, trace=False)
            for kk, v in im.items():
                sim.tensor(kk)[:] = v
            sim.simulate()
            results.append({o: np.array(sim.mem_tensor(o))
                            for o in ("OA", "OB", "OC", "OS")})
        last_result = None
    else:
        res = run_bass_kernel_spmd(nc, in_maps, core_ids=list(range(N_CORES)))
        last_result = res
        results = res.results

    # ---- assemble output -------------------------------------------------
    OUT = np.empty((BATCH, DTOT), dtype=np.float32)

    def unslot(seg, cap, d):
        # device slot i -> [i % 128, i // 128]; seg is [128, (cap//128)*d]
        seg = np.asarray(seg).astype(np.float32)
        return seg.reshape(128, cap // 128, d).transpose(1, 0, 2).reshape(cap, d)

    for k in range(N_CORES):
        oa = np.asarray(results[k]["OA"])
        a_off = 0
        for s in range(N_RANGES):
            mc = capsA[s] // 128
            rows = unslot(oa[:, a_off * 64:(a_off + mc) * 64], capsA[s], 64)
            a_off += mc
            _, b, j = bucketsA[k][s]
            n = len(b)
            for jj, col in enumerate(A_COLS):
                sel = j == jj
                OUT[b[sel], OFFS[col]:OFFS[col] + 64] = rows[:n][sel]
        rowsB = unslot(results[k]["OB"], capB, 32)
        b, j = destB[k]
        n = len(b)
        for jj, col in enumerate(B_COLS):
            sel = j == jj
            OUT[b[sel], OFFS[col]:OFFS[col] + 32] = rowsB[:n][sel]
        # OC: [128, c(3), t(16), 16] with slot-col j = c*16 + t; slot order is
        # the per-column sorted order, so scatter back through permCS
        oc = np.asarray(results[k]["OC"]).astype(np.float32)
        oc = oc.reshape(128, 3, 16, 16)
        for j, col in enumerate(CS_GCOLS):
            blk = oc[:, j].transpose(1, 0, 2).reshape(BPC, 16)
            OUT[k * BPC + permCS[k][j], OFFS[col]:OFFS[col] + 16] = blk
        # OS: matmul tile t partition p -> batch row t*128+p (identity)
        os_ = np.asarray(results[k]["OS"]).astype(np.float32)
        os_ = os_.reshape(128, 16, 8).transpose(1, 0, 2).reshape(BPC, 8)
        OUT[k * BPC:(k + 1) * BPC, OFFS[7]:OFFS[7] + 8] = os_
    return OUT
